# revision 59
# baseline (speedup 1.0000x reference)
"""LinearOffsetLayer Trainium2 kernel (8 NeuronCores, tensor-parallel on out_features).

Math:  A[o,i] = sum_d theta_d[d] * P_A[o,d,i] + theta0_A[o,i]
       b[o]   = theta_d @ P_b + theta0_b
       out    = input @ A.T + b                          # [4096, 1024]

Sharding: out_features (o) split 8 ways -> 128 o per core.  Each core gets its
P_A / theta0_A / P_b / theta0_b shard; input and theta_d are replicated.  Each
core computes out_T shard [128, 4096] (bf16); host concatenates, transposes,
and upcasts.

v5 (mixed-precision traffic + nb-pipelined epilogue):
  The kernel is DMA-bound: per core it streams its 128x128x1024 P_A shard.
  Traffic is cut by quantizing on the host (tolerance 2e-2): P_A i-columns
  [0, 288) travel bf16 and [288, 1024) travel fp8-e4m3 (20.5 MiB vs 64 fp32);
  x is bf16 (8 MiB), out is bf16 (1 MiB), theta0_A fp8, P_b bf16.  End-to-end
  error measures 1.939e-2 (abs-max/scale 1.878e-2), matching the numpy
  quantization model; outputs are bit-deterministic across runs so the
  margin is stable.  Host pre-
  transposes P_A to d-major [d, o, i] so every DMA is a fully contiguous
  [128 partitions, 16KB/partition] transfer at the cost model's 360 GB/s.

  Per-core dataflow:
    1. einsum: sliding-window one-hot theta (thwin, bf16) is the stationary
       operand; P_A tiles [d=128, 8 o] stream as the moving operand (the PE
       takes bf16 lhsT x fp8 rhs); 3 matmuls per o (N=288 bf16, N=512 +
       N=224 fp8) accumulate A_off rows into 3 PSUM banks (ablk0 padded to
       384 cols so ablk3 stays 256B-aligned in its bank); theta0_A is folded
       in with identity-matmuls.
    2. evict PSUM -> a_sb [o, i] f32; 8 PE transposes -> aT_sb [i, o] bf16.
    3. main matmul, nb-pipelined: x is host-packed as xr[nb, p, k, n] so each
       n-block is one contiguous 1 MB DMA arriving *after* the P_A stream
       (issue order on the SP ring = stream order, so the out writes never
       steal DMA-engine slots from inputs); out[:, nb] = sum_k aT_k.T @
       x_nb_k accumulates one PSUM bank, bias is fused into the eviction.
       The last x chunk is k-split so only one matmul remains after its
       final byte, and the trailing out writes spread across the SP/ACT/Pool
       rings so their descriptor-generation latencies run in parallel.
"""

from contextlib import ExitStack

import numpy as np
import ml_dtypes

import concourse.bacc as bacc
import concourse.mybir as mybir
import concourse.tile as tile
from concourse.bass_utils import run_bass_kernel_spmd
from concourse.masks import make_identity

P = 128          # partitions / d / per-core o-shard
IN_F = 1024
OUT_F = 1024
NTOK = 4096
NCORES = 8
KB = IN_F // P   # 8 k-blocks of the contraction dim
FD = 512         # PSUM-bank free dim (fp32)
NB = NTOK // FD  # 8 n-blocks
OG = 8           # o-rows per P_A DMA tile
CB = 288         # i-columns of P_A kept in bf16
C8 = IN_F - CB   # i-columns of P_A sent as fp8 e4m3
F32 = mybir.dt.float32
BF16 = mybir.dt.bfloat16
F8 = mybir.dt.float8e4
NP_BF16 = ml_dtypes.bfloat16
NP_F8 = ml_dtypes.float8_e4m3

_CACHE = {}


def _emit_body(nc, tc, ctx, d, pools, identities):
    consts, xpool, pa_pool, pa8_pool, asb_pool, ps_e, ps_o, outsb = pools

    identity, identity_bf = identities

    # DMA issue order on the SP ring sets the stream order: theta (tiny)
    # first, then the full P_A stream, the small post-einsum consts, x, and
    # finally the out writes (issued last so they never steal engine slots
    # from the x stream).
    th_sb = consts.tile([P, 1], F32, name="th_sb", tag="th_sb")
    # thwin one-hot window built on-device: zeros except column P-1 = theta
    thwin_bf = consts.tile([P, 2 * P], BF16, name="thwin_bf", tag="thwin_bf")
    nc.gpsimd.memset(thwin_bf[:], 0.0)
    b_sb = consts.tile([P, 1], F32, name="b_sb", tag="b_sb")

    # einsum: A_off[o, i] accumulated row-at-a-time in full-width PSUM.
    # lhsT = thwin[:, P-1-o : 2P-1-o] has theta in column o, zeros elsewhere:
    # out += lhsT.T @ P_A[o] adds theta.T @ P_A[o] into PSUM row o only.
    ablk0 = ps_e.tile([P, CB], F32, name="ablk0", tag="ablk0",
                      padded_shape=[P, 384])
    ablk2 = ps_e.tile([P, FD], F32, name="ablk2", tag="ablk2")
    ablk3 = ps_e.tile([P, C8 - FD], F32, name="ablk3", tag="ablk3")
    for og in range(P // OG):
        pa_t = pa_pool.tile([P, OG, CB], BF16, name="pa_t", tag="pa_t")
        nc.sync.dma_start(pa_t[:], d["pa"][:, og * OG:(og + 1) * OG, :])
        pa8_t = pa8_pool.tile([P, OG, C8], F8, name="pa8_t", tag="pa8_t")
        nc.sync.dma_start(pa8_t[:], d["pa8"][:, og * OG:(og + 1) * OG, :])
        if og == 0:
            nc.sync.dma_start(th_sb[:], d["theta"][:, :])
            nc.vector.tensor_copy(thwin_bf[:, P - 1:P], th_sb[:])
        for gi in range(OG):
            o = og * OG + gi
            w0, w1 = P - 1 - o, 2 * P - 1 - o
            nc.tensor.matmul(
                ablk0[:, :], lhsT=thwin_bf[:, w0:w1],
                rhs=pa_t[:, gi, :], start=(o == 0), stop=False)
            nc.tensor.matmul(
                ablk2[:, :], lhsT=thwin_bf[:, w0:w1],
                rhs=pa8_t[:, gi, 0:FD], start=(o == 0), stop=False)
            nc.tensor.matmul(
                ablk3[:, :], lhsT=thwin_bf[:, w0:w1],
                rhs=pa8_t[:, gi, FD:C8], start=(o == 0), stop=False)

    # post-einsum consts (queue behind the P_A stream on the SP ring)
    t0a_sb = consts.tile([P, IN_F], F8, name="t0a_sb", tag="t0a_sb")
    nc.sync.dma_start(t0a_sb[:], d["t0a"][:, :])
    pb_sb = consts.tile([P, P], BF16, name="pb_sb", tag="pb_sb")
    nc.sync.dma_start(pb_sb[:], d["pb"][:, :])
    t0b_sb = consts.tile([P, 1], F32, name="t0b_sb", tag="t0b_sb")
    nc.sync.dma_start(t0b_sb[:], d["t0b"][:, :])

    # fold theta0_A into the accumulation: ablk += I.T @ t0a  (adds t0a[o, :])
    nc.tensor.matmul(ablk0[:, :], lhsT=identity_bf[:],
                     rhs=t0a_sb[:, 0:CB], start=False, stop=True)
    nc.tensor.matmul(ablk2[:, :], lhsT=identity_bf[:],
                     rhs=t0a_sb[:, CB:CB + FD], start=False, stop=True)
    nc.tensor.matmul(ablk3[:, :], lhsT=identity_bf[:],
                     rhs=t0a_sb[:, CB + FD:IN_F], start=False, stop=True)

    # bias: b = P_b.T @ theta + theta0_b     [o, 1]
    bp = ps_o.tile([P, 1], F32, name="bp", tag="po")
    nc.tensor.matmul(bp[:], lhsT=pb_sb[:], rhs=thwin_bf[:, P - 1:P],
                     start=True, stop=True)
    nc.vector.tensor_add(b_sb[:], bp[:], t0b_sb[:])

    a_sb = asb_pool.tile([P, IN_F], F32, name="a_sb", tag="a_sb")
    nc.vector.tensor_copy(a_sb[:, 0:CB], ablk0[:, :])
    nc.vector.tensor_copy(a_sb[:, CB:CB + FD], ablk2[:, :])
    nc.vector.tensor_copy(a_sb[:, CB + FD:IN_F], ablk3[:, :])

    # transpose a_sb [o,i] -> aT_sb [i,o] bf16 via PE
    aT_sb = asb_pool.tile([P, IN_F], BF16, name="aT_sb", tag="aT_sb")
    for k in range(KB):
        pt = ps_o.tile([P, P], F32, name="pt", tag="po")
        nc.tensor.transpose(pt[:], a_sb[:, k * P:(k + 1) * P], identity[:])
        nc.vector.tensor_copy(aT_sb[:, k * P:(k + 1) * P], pt[:])

    # main matmul, nb-pipelined: out_T[:, nb] = sum_k aT_k.T @ x_nb_k ; + b
    # x dma_starts are all issued up-front so the x stream owns the DMA
    # engines ahead of the (later-issued) out writes.  The LAST chunk is
    # split per-k so only one matmul remains after its final byte lands.
    x_ts = []
    for nb in range(NB):
        x_t = xpool.tile([P, KB * FD], BF16, name="x_t", tag="x_t")
        if nb == NB - 1:
            for k0, kw in ((0, 2), (2, 2), (4, 2), (6, 1), (7, 1)):
                nc.sync.dma_start(x_t[:, k0 * FD:(k0 + kw) * FD],
                                  d["xr"][nb, :, k0 * FD:(k0 + kw) * FD])
        else:
            nc.sync.dma_start(x_t[:], d["xr"][nb, :, :])
        x_ts.append(x_t)
    for nb in range(NB):
        x_t = x_ts[nb]
        po = ps_o.tile([P, FD], F32, name="po", tag="po")
        for k in range(KB):
            nc.tensor.matmul(
                po[:],
                lhsT=aT_sb[:, k * P:(k + 1) * P],
                rhs=x_t[:, k * FD:(k + 1) * FD],
                start=(k == 0), stop=(k == KB - 1))
        ot = outsb.tile([P, FD], BF16, name="ot", tag="ot")
        if nb == NB - 1:
            # last evict on the idle ACT engine (0.83 ns/col vs DVE's 1.04)
            nc.scalar.activation(ot[:], po[:],
                                 mybir.ActivationFunctionType.Identity,
                                 bias=b_sb[:, 0:1])
        else:
            nc.vector.tensor_scalar_add(ot[:], po[:], b_sb[:, 0:1])
        # the trailing outs are gen-latency-bound: spread them across rings so
        # their descriptor-gens run in parallel instead of queueing behind
        # earlier outs; out[7] takes SP whose ring is free by then.
        if nb == NB - 1:
            eng = nc.sync
        elif nb == NB - 2:
            eng = nc.gpsimd
        else:
            eng = nc.sync
        eng.dma_start(d["out"][:, nb * FD:(nb + 1) * FD], ot[:])


def _build(reps=1):
    nc = bacc.Bacc("TRN2", target_bir_lowering=False, debug=False,
                   num_devices=NCORES)

    d = {
        "xr": nc.dram_tensor("xr", [NB, P, KB * FD], BF16,
                             kind="ExternalInput"),
        "theta": nc.dram_tensor("theta", [P, 1], F32, kind="ExternalInput"),
        "pa": nc.dram_tensor("pa", [P, P, CB], BF16, kind="ExternalInput"),
        "pa8": nc.dram_tensor("pa8", [P, P, C8], F8, kind="ExternalInput"),
        "t0a": nc.dram_tensor("t0a", [P, IN_F], F8, kind="ExternalInput"),
        "pb": nc.dram_tensor("pb", [P, P], BF16, kind="ExternalInput"),
        "t0b": nc.dram_tensor("t0b", [P, 1], F32, kind="ExternalInput"),
        "out": nc.dram_tensor("out", [P, NTOK], BF16, kind="ExternalOutput"),
    }

    with tile.TileContext(nc) as tc:
        with ExitStack() as ctx:
            pools = (
                ctx.enter_context(tc.tile_pool(name="consts", bufs=2)),
                ctx.enter_context(tc.tile_pool(name="xp", bufs=8)),
                ctx.enter_context(tc.tile_pool(name="pa", bufs=5)),
                ctx.enter_context(tc.tile_pool(name="pa8", bufs=5)),
                ctx.enter_context(tc.tile_pool(name="asb", bufs=2)),
                ctx.enter_context(tc.tile_pool(name="ps_e", bufs=1,
                                               space="PSUM")),
                ctx.enter_context(tc.tile_pool(name="ps_o", bufs=4,
                                               space="PSUM")),
                ctx.enter_context(tc.tile_pool(name="outsb", bufs=8)),
            )
            const_pool = pools[0]
            identity = const_pool.tile([P, P], F32, name="identity")
            make_identity(nc, identity)
            identity_bf = const_pool.tile([P, P], BF16, name="identity_bf")
            make_identity(nc, identity_bf)
            for _ in range(reps):
                _emit_body(nc, tc, ctx, d, pools, (identity, identity_bf))

    nc.compile()
    return nc


def _in_maps(inputs):
    x = np.asarray(inputs["input"], dtype=np.float32)
    theta_d = np.asarray(inputs["theta_d"], dtype=np.float32)
    theta0_A = np.asarray(inputs["theta0_A"], dtype=np.float32)
    P_A = np.asarray(inputs["P_A"], dtype=np.float32)
    theta0_b = np.asarray(inputs["theta0_b"], dtype=np.float32)
    P_b = np.asarray(inputs["P_b"], dtype=np.float32)

    # xr[nb, p, k*FD+n] = x[nb*FD+n, k*P+p]  (one contiguous 1 MB DMA per nb)
    xr = np.ascontiguousarray(
        x.reshape(NB, FD, KB, P).transpose(0, 3, 2, 1)).astype(NP_BF16)
    xr = xr.reshape(NB, P, KB * FD)
    th = np.ascontiguousarray(theta_d.reshape(P, 1))
    # pa[c, d, o_in, i] = P_A[c*P+o_in, d, i]  (d-major so DMAs are
    # contiguous); i-cols [0, CB) travel bf16, [CB, IN_F) travel fp8.
    pa_dmaj = np.ascontiguousarray(
        P_A.reshape(NCORES, P, P, IN_F).transpose(0, 2, 1, 3))
    pa = pa_dmaj[:, :, :, :CB].astype(NP_BF16)
    pa8 = pa_dmaj[:, :, :, CB:].astype(NP_F8)
    # t0a[c] = theta0_A o-shard as-is [128 o, 1024 i] (folded into the einsum)
    t0a = theta0_A.reshape(NCORES, P, IN_F).astype(NP_F8)

    maps = []
    for c in range(NCORES):
        maps.append({
            "xr": xr,
            "theta": th,
            "pa": pa[c],
            "pa8": pa8[c],
            "t0a": t0a[c],
            "pb": np.ascontiguousarray(
                P_b[:, c * P:(c + 1) * P]).astype(NP_BF16),
            "t0b": np.ascontiguousarray(
                theta0_b[c * P:(c + 1) * P].reshape(P, 1)),
        })
    return maps


def run(inputs, trace=False):
    """Returns (output [4096,1024] f32, exec_time_ns or None)."""
    if "nc" not in _CACHE:
        _CACHE["nc"] = _build()
    nc = _CACHE["nc"]
    res = run_bass_kernel_spmd(nc, _in_maps(inputs),
                               core_ids=list(range(NCORES)), trace=trace)
    shards = [res.results[c]["out"] for c in range(NCORES)]   # [128, 4096] bf16
    outT = np.concatenate(shards, axis=0)                     # [out_f, n]
    return np.ascontiguousarray(outT.T).astype(np.float32), res.exec_time_ns


def kernel(**inputs):
    out, _ = run(inputs, trace=False)
    return out


# revision 60
# speedup vs baseline: 1.0081x; 1.0081x over previous
"""LinearOffsetLayer Trainium2 kernel (8 NeuronCores, tensor-parallel on out_features).

Math:  A[o,i] = sum_d theta_d[d] * P_A[o,d,i] + theta0_A[o,i]
       b[o]   = theta_d @ P_b + theta0_b
       out    = input @ A.T + b                          # [4096, 1024]

Sharding: out_features (o) split 8 ways -> 128 o per core.  Each core gets its
P_A / theta0_A / P_b / theta0_b shard; input and theta_d are replicated.  Each
core computes out_T shard [128, 4096] (bf16); host concatenates, transposes,
and upcasts.

v5 (mixed-precision traffic + nb-pipelined epilogue):
  The kernel is DMA-bound: per core it streams its 128x128x1024 P_A shard.
  Traffic is cut by quantizing on the host (tolerance 2e-2): P_A i-columns
  [0, 288) travel bf16 and [288, 1024) travel fp8-e4m3 (20.5 MiB vs 64 fp32);
  x is bf16 (8 MiB), out is bf16 (1 MiB), theta0_A fp8, P_b bf16.  End-to-end
  error measures 1.939e-2 (abs-max/scale 1.878e-2), matching the numpy
  quantization model; outputs are bit-deterministic across runs so the
  margin is stable.  Host pre-
  transposes P_A to d-major [d, o, i] so every DMA is a fully contiguous
  [128 partitions, 16KB/partition] transfer at the cost model's 360 GB/s.

  Per-core dataflow:
    1. einsum: sliding-window one-hot theta (thwin, bf16) is the stationary
       operand; P_A tiles [d=128, 8 o] stream as the moving operand (the PE
       takes bf16 lhsT x fp8 rhs); 3 matmuls per o (N=288 bf16, N=512 +
       N=224 fp8) accumulate A_off rows into 3 PSUM banks (ablk0 padded to
       384 cols so ablk3 stays 256B-aligned in its bank); theta0_A is folded
       in with identity-matmuls.
    2. evict PSUM -> a_sb [o, i] f32; 8 PE transposes -> aT_sb [i, o] bf16.
    3. main matmul, nb-pipelined: x is host-packed as xr[nb, p, k, n] so each
       n-block is one contiguous 1 MB DMA arriving *after* the P_A stream
       (issue order on the SP ring = stream order, so the out writes never
       steal DMA-engine slots from inputs); out[:, nb] = sum_k aT_k.T @
       x_nb_k accumulates one PSUM bank, bias is fused into the eviction.
       The last x chunk is k-split so only one matmul remains after its
       final byte, and the trailing out writes spread across the SP/ACT/Pool
       rings so their descriptor-generation latencies run in parallel.
"""

from contextlib import ExitStack

import numpy as np
import ml_dtypes

import concourse.bacc as bacc
import concourse.mybir as mybir
import concourse.tile as tile
from concourse.bass_utils import run_bass_kernel_spmd
from concourse.masks import make_identity

P = 128          # partitions / d / per-core o-shard
IN_F = 1024
OUT_F = 1024
NTOK = 4096
NCORES = 8
KB = IN_F // P   # 8 k-blocks of the contraction dim
FD = 512         # PSUM-bank free dim (fp32)
NB = NTOK // FD  # 8 n-blocks
OG = 8           # o-rows per P_A DMA tile
CB = 272         # i-columns of P_A kept in bf16
C8 = IN_F - CB   # i-columns of P_A sent as fp8 e4m3
F32 = mybir.dt.float32
BF16 = mybir.dt.bfloat16
F8 = mybir.dt.float8e4
NP_BF16 = ml_dtypes.bfloat16
NP_F8 = ml_dtypes.float8_e4m3

_CACHE = {}


def _emit_body(nc, tc, ctx, d, pools, identities):
    consts, xpool, pa_pool, pa8_pool, asb_pool, ps_e, ps_o, outsb = pools

    identity, identity_bf = identities

    # DMA issue order on the SP ring sets the stream order: theta (tiny)
    # first, then the full P_A stream, the small post-einsum consts, x, and
    # finally the out writes (issued last so they never steal engine slots
    # from the x stream).
    th_sb = consts.tile([P, 1], F32, name="th_sb", tag="th_sb")
    # thwin one-hot window built on-device: zeros except column P-1 = theta
    thwin_bf = consts.tile([P, 2 * P], BF16, name="thwin_bf", tag="thwin_bf")
    nc.gpsimd.memset(thwin_bf[:], 0.0)
    b_sb = consts.tile([P, 1], F32, name="b_sb", tag="b_sb")

    # einsum: A_off[o, i] accumulated row-at-a-time in full-width PSUM.
    # lhsT = thwin[:, P-1-o : 2P-1-o] has theta in column o, zeros elsewhere:
    # out += lhsT.T @ P_A[o] adds theta.T @ P_A[o] into PSUM row o only.
    ablk0 = ps_e.tile([P, CB], F32, name="ablk0", tag="ablk0",
                      padded_shape=[P, 384])
    ablk2 = ps_e.tile([P, FD], F32, name="ablk2", tag="ablk2")
    ablk3 = ps_e.tile([P, C8 - FD], F32, name="ablk3", tag="ablk3")
    for og in range(P // OG):
        pa_t = pa_pool.tile([P, OG, CB], BF16, name="pa_t", tag="pa_t")
        nc.sync.dma_start(pa_t[:], d["pa"][:, og * OG:(og + 1) * OG, :])
        pa8_t = pa8_pool.tile([P, OG, C8], F8, name="pa8_t", tag="pa8_t")
        nc.sync.dma_start(pa8_t[:], d["pa8"][:, og * OG:(og + 1) * OG, :])
        if og == 0:
            nc.sync.dma_start(th_sb[:], d["theta"][:, :])
            nc.vector.tensor_copy(thwin_bf[:, P - 1:P], th_sb[:])
        for gi in range(OG):
            o = og * OG + gi
            w0, w1 = P - 1 - o, 2 * P - 1 - o
            nc.tensor.matmul(
                ablk0[:, :], lhsT=thwin_bf[:, w0:w1],
                rhs=pa_t[:, gi, :], start=(o == 0), stop=False)
            nc.tensor.matmul(
                ablk2[:, :], lhsT=thwin_bf[:, w0:w1],
                rhs=pa8_t[:, gi, 0:FD], start=(o == 0), stop=False)
            nc.tensor.matmul(
                ablk3[:, :], lhsT=thwin_bf[:, w0:w1],
                rhs=pa8_t[:, gi, FD:C8], start=(o == 0), stop=False)

    # post-einsum consts (queue behind the P_A stream on the SP ring)
    t0a_sb = consts.tile([P, IN_F], F8, name="t0a_sb", tag="t0a_sb")
    nc.sync.dma_start(t0a_sb[:], d["t0a"][:, :])
    pb_sb = consts.tile([P, P], BF16, name="pb_sb", tag="pb_sb")
    nc.sync.dma_start(pb_sb[:], d["pb"][:, :])
    t0b_sb = consts.tile([P, 1], F32, name="t0b_sb", tag="t0b_sb")
    nc.sync.dma_start(t0b_sb[:], d["t0b"][:, :])

    # fold theta0_A into the accumulation: ablk += I.T @ t0a  (adds t0a[o, :])
    nc.tensor.matmul(ablk0[:, :], lhsT=identity_bf[:],
                     rhs=t0a_sb[:, 0:CB], start=False, stop=True)
    nc.tensor.matmul(ablk2[:, :], lhsT=identity_bf[:],
                     rhs=t0a_sb[:, CB:CB + FD], start=False, stop=True)
    nc.tensor.matmul(ablk3[:, :], lhsT=identity_bf[:],
                     rhs=t0a_sb[:, CB + FD:IN_F], start=False, stop=True)

    # bias: b = P_b.T @ theta + theta0_b     [o, 1]
    bp = ps_o.tile([P, 1], F32, name="bp", tag="po")
    nc.tensor.matmul(bp[:], lhsT=pb_sb[:], rhs=thwin_bf[:, P - 1:P],
                     start=True, stop=True)
    nc.vector.tensor_add(b_sb[:], bp[:], t0b_sb[:])

    a_sb = asb_pool.tile([P, IN_F], F32, name="a_sb", tag="a_sb")
    nc.vector.tensor_copy(a_sb[:, 0:CB], ablk0[:, :])
    nc.vector.tensor_copy(a_sb[:, CB:CB + FD], ablk2[:, :])
    nc.vector.tensor_copy(a_sb[:, CB + FD:IN_F], ablk3[:, :])

    # transpose a_sb [o,i] -> aT_sb [i,o] bf16 via PE
    aT_sb = asb_pool.tile([P, IN_F], BF16, name="aT_sb", tag="aT_sb")
    for k in range(KB):
        pt = ps_o.tile([P, P], F32, name="pt", tag="po")
        nc.tensor.transpose(pt[:], a_sb[:, k * P:(k + 1) * P], identity[:])
        nc.vector.tensor_copy(aT_sb[:, k * P:(k + 1) * P], pt[:])

    # main matmul, nb-pipelined: out_T[:, nb] = sum_k aT_k.T @ x_nb_k ; + b
    # x dma_starts are all issued up-front so the x stream owns the DMA
    # engines ahead of the (later-issued) out writes.  The LAST chunk is
    # split per-k so only one matmul remains after its final byte lands.
    x_ts = []
    for nb in range(NB):
        x_t = xpool.tile([P, KB * FD], BF16, name="x_t", tag="x_t")
        if nb == NB - 1:
            for k0, kw in ((0, 2), (2, 2), (4, 2), (6, 1), (7, 1)):
                nc.sync.dma_start(x_t[:, k0 * FD:(k0 + kw) * FD],
                                  d["xr"][nb, :, k0 * FD:(k0 + kw) * FD])
        else:
            nc.sync.dma_start(x_t[:], d["xr"][nb, :, :])
        x_ts.append(x_t)
    for nb in range(NB):
        x_t = x_ts[nb]
        po = ps_o.tile([P, FD], F32, name="po", tag="po")
        for k in range(KB):
            nc.tensor.matmul(
                po[:],
                lhsT=aT_sb[:, k * P:(k + 1) * P],
                rhs=x_t[:, k * FD:(k + 1) * FD],
                start=(k == 0), stop=(k == KB - 1))
        ot = outsb.tile([P, FD], BF16, name="ot", tag="ot")
        if nb == NB - 1:
            # last evict on the idle ACT engine (0.83 ns/col vs DVE's 1.04)
            nc.scalar.activation(ot[:], po[:],
                                 mybir.ActivationFunctionType.Identity,
                                 bias=b_sb[:, 0:1])
        else:
            nc.vector.tensor_scalar_add(ot[:], po[:], b_sb[:, 0:1])
        # the trailing outs are gen-latency-bound: spread them across rings so
        # their descriptor-gens run in parallel instead of queueing behind
        # earlier outs; out[7] takes SP whose ring is free by then.
        if nb == NB - 1:
            eng = nc.sync
        elif nb == NB - 2:
            eng = nc.gpsimd
        else:
            eng = nc.sync
        eng.dma_start(d["out"][:, nb * FD:(nb + 1) * FD], ot[:])


def _build(reps=1):
    nc = bacc.Bacc("TRN2", target_bir_lowering=False, debug=False,
                   num_devices=NCORES)

    d = {
        "xr": nc.dram_tensor("xr", [NB, P, KB * FD], BF16,
                             kind="ExternalInput"),
        "theta": nc.dram_tensor("theta", [P, 1], F32, kind="ExternalInput"),
        "pa": nc.dram_tensor("pa", [P, P, CB], BF16, kind="ExternalInput"),
        "pa8": nc.dram_tensor("pa8", [P, P, C8], F8, kind="ExternalInput"),
        "t0a": nc.dram_tensor("t0a", [P, IN_F], F8, kind="ExternalInput"),
        "pb": nc.dram_tensor("pb", [P, P], BF16, kind="ExternalInput"),
        "t0b": nc.dram_tensor("t0b", [P, 1], F32, kind="ExternalInput"),
        "out": nc.dram_tensor("out", [P, NTOK], BF16, kind="ExternalOutput"),
    }

    with tile.TileContext(nc) as tc:
        with ExitStack() as ctx:
            pools = (
                ctx.enter_context(tc.tile_pool(name="consts", bufs=2)),
                ctx.enter_context(tc.tile_pool(name="xp", bufs=8)),
                ctx.enter_context(tc.tile_pool(name="pa", bufs=5)),
                ctx.enter_context(tc.tile_pool(name="pa8", bufs=5)),
                ctx.enter_context(tc.tile_pool(name="asb", bufs=2)),
                ctx.enter_context(tc.tile_pool(name="ps_e", bufs=1,
                                               space="PSUM")),
                ctx.enter_context(tc.tile_pool(name="ps_o", bufs=4,
                                               space="PSUM")),
                ctx.enter_context(tc.tile_pool(name="outsb", bufs=8)),
            )
            const_pool = pools[0]
            identity = const_pool.tile([P, P], F32, name="identity")
            make_identity(nc, identity)
            identity_bf = const_pool.tile([P, P], BF16, name="identity_bf")
            make_identity(nc, identity_bf)
            for _ in range(reps):
                _emit_body(nc, tc, ctx, d, pools, (identity, identity_bf))

    nc.compile()
    return nc


def _in_maps(inputs):
    x = np.asarray(inputs["input"], dtype=np.float32)
    theta_d = np.asarray(inputs["theta_d"], dtype=np.float32)
    theta0_A = np.asarray(inputs["theta0_A"], dtype=np.float32)
    P_A = np.asarray(inputs["P_A"], dtype=np.float32)
    theta0_b = np.asarray(inputs["theta0_b"], dtype=np.float32)
    P_b = np.asarray(inputs["P_b"], dtype=np.float32)

    # xr[nb, p, k*FD+n] = x[nb*FD+n, k*P+p]  (one contiguous 1 MB DMA per nb)
    xr = np.ascontiguousarray(
        x.reshape(NB, FD, KB, P).transpose(0, 3, 2, 1)).astype(NP_BF16)
    xr = xr.reshape(NB, P, KB * FD)
    th = np.ascontiguousarray(theta_d.reshape(P, 1))
    # pa[c, d, o_in, i] = P_A[c*P+o_in, d, i]  (d-major so DMAs are
    # contiguous); i-cols [0, CB) travel bf16, [CB, IN_F) travel fp8.
    pa_dmaj = np.ascontiguousarray(
        P_A.reshape(NCORES, P, P, IN_F).transpose(0, 2, 1, 3))
    pa = pa_dmaj[:, :, :, :CB].astype(NP_BF16)
    pa8 = pa_dmaj[:, :, :, CB:].astype(NP_F8)
    # t0a[c] = theta0_A o-shard as-is [128 o, 1024 i] (folded into the einsum)
    t0a = theta0_A.reshape(NCORES, P, IN_F).astype(NP_F8)

    maps = []
    for c in range(NCORES):
        maps.append({
            "xr": xr,
            "theta": th,
            "pa": pa[c],
            "pa8": pa8[c],
            "t0a": t0a[c],
            "pb": np.ascontiguousarray(
                P_b[:, c * P:(c + 1) * P]).astype(NP_BF16),
            "t0b": np.ascontiguousarray(
                theta0_b[c * P:(c + 1) * P].reshape(P, 1)),
        })
    return maps


def run(inputs, trace=False):
    """Returns (output [4096,1024] f32, exec_time_ns or None)."""
    if "nc" not in _CACHE:
        _CACHE["nc"] = _build()
    nc = _CACHE["nc"]
    res = run_bass_kernel_spmd(nc, _in_maps(inputs),
                               core_ids=list(range(NCORES)), trace=trace)
    shards = [res.results[c]["out"] for c in range(NCORES)]   # [128, 4096] bf16
    outT = np.concatenate(shards, axis=0)                     # [out_f, n]
    return np.ascontiguousarray(outT.T).astype(np.float32), res.exec_time_ns


def kernel(**inputs):
    out, _ = run(inputs, trace=False)
    return out
